# revision 3
# baseline (speedup 1.0000x reference)
"""Trainium2 Bass kernel for nn_ConceptIntergation (histogram_binning).

Reference computation:
    counts[b,s,n] = sum_k one_hot(concepts[b,s,k], 129)[..., n]  (n < 128; 128 = padding)
    out[b,s,n,d]  = counts[b,s,n] * emb_table[n,d]

Strategy (data-parallel over batch, 8 cores; HBM-write roofline ~146us/core):
  - Each core handles B_LOC=8 batches -> 1600 (b,s) rows, output shard
    [1600, 128*64] f32 (~52 MB). Only the tiny inputs (idx 27KB, emb 32KB,
    iota/ident 130KB) are read from HBM -- no 4MB pre-broadcast table.
  - The expansion out_block[rows, (n d)] = counts @ W is computed on the
    TENSOR engine: W[n', n*64+d] = emb[n,d] * (n==n') is a block-diagonal
    [128, 8192] bf16 matrix built on-device (GpSimd affine_select + one DVE
    tensor_tensor), and lhsT = countsT (PE transpose of the DVE histogram).
    Exactly one nonzero product per output element, so bf16 only rounds
    emb (rel err <= 2^-8), accumulated in f32 PSUM.
  - PSUM->SBUF drains alternate ScalarE/VectorE; stores are 1MB
    [128, 2048] chunks. Engine budgets per core: PE ~50us (cold), DVE
    ~50us, ScalarE ~35us, GpSimd ~10us -- all far under the 146us DMA
    floor, so the store stream saturates from ~4us to the end.
"""

import numpy as np

import concourse.bass as bass
import concourse.mybir as mybir
from concourse import bacc
from concourse.tile import TileContext
from concourse.bass_utils import run_bass_kernel_spmd

B, S, K = 64, 200, 4
N, D = 128, 64
ND = N * D                      # 8192
NCORES = 8
B_LOC = B // NCORES             # 8
ROWS = B_LOC * S                # 1600 (b,s) rows per core
P = 128
NBLK = (ROWS + P - 1) // P      # 13 (12 full + 1 of 64 rows)

CC = 4                          # W chunks / output column stripes
CW = ND // CC                   # 2048 cols per stripe (1 MB stores)
MW = CW // D                    # 32 n-rows per stripe
FD = 512                        # matmul moving free dim (1 PSUM bank f32)

F32 = mybir.dt.float32
BF16 = mybir.dt.bfloat16

_NC_CACHE = {}


def _build_nc():
    nc = bacc.Bacc()
    idx = nc.declare_dram_parameter("idx", [P, NBLK * K], F32, isOutput=False)
    emb = nc.declare_dram_parameter("emb", [N, D], F32, isOutput=False)
    iota = nc.declare_dram_parameter("iota", [P, N], F32, isOutput=False)
    ident = nc.declare_dram_parameter("ident", [P, P], F32, isOutput=False)
    out = nc.declare_dram_parameter("out", [ROWS, ND], F32, isOutput=True)

    with TileContext(nc) as tc:
        with (
            tc.tile_pool(name="const", bufs=1) as cpool,
            tc.tile_pool(name="cnt", bufs=2) as cntpool,
            tc.tile_pool(name="cntT", bufs=NBLK) as ctpool,
            tc.tile_pool(name="work", bufs=12) as wpool,
            tc.tile_pool(name="psmm", bufs=3, space="PSUM") as pmm,
            tc.tile_pool(name="pstr", bufs=2, space="PSUM") as ptr,
        ):
            iota_sb = cpool.tile([P, N], F32)
            nc.sync.dma_start(out=iota_sb, in_=iota[:, :])
            idx_sb = cpool.tile([P, NBLK * K], F32)
            nc.sync.dma_start(out=idx_sb, in_=idx[:, :])
            emb_sb = cpool.tile([N, D], F32)
            nc.sync.dma_start(out=emb_sb, in_=emb[:, :])
            ident_sb = cpool.tile([P, P], F32)
            nc.sync.dma_start(out=ident_sb, in_=ident[:, :])

            # Block-diagonal weight W[n', m*64+d] = emb[n',d] * (n' == cc*32+m),
            # built on-device. Chunk 0 on DVE (fast, gates the first matmuls);
            # chunks 1..3 on the otherwise-idle GpSimd.
            Wt = [
                cpool.tile([P, CW], BF16, tag=f"W{c}", name=f"W{c}")
                for c in range(CC)
            ]
            emb_bc = emb_sb[:, None, :].broadcast_to([P, MW, D])
            nc.vector.tensor_tensor(
                out=Wt[0].rearrange("p (m d) -> p m d", d=D),
                in0=emb_bc,
                in1=ident_sb[:, 0:MW, None].broadcast_to([P, MW, D]),
                op=mybir.AluOpType.mult,
            )
            for c in range(1, CC):
                nc.gpsimd.affine_select(
                    out=Wt[c].rearrange("p (m d) -> p m d", d=D),
                    in_=emb_bc,
                    pattern=[[1, MW], [0, D]],
                    compare_op=mybir.AluOpType.is_equal,
                    fill=0.0,
                    base=c * MW,
                    channel_multiplier=-1,
                )

            def emit_hist(j, counts, pj):
                nc.vector.tensor_scalar(
                    out=counts[:pj],
                    in0=iota_sb[:pj],
                    scalar1=idx_sb[:pj, j * K : j * K + 1],
                    scalar2=None,
                    op0=mybir.AluOpType.is_equal,
                )
                for k in range(1, K):
                    nc.vector.scalar_tensor_tensor(
                        out=counts[:pj],
                        in0=iota_sb[:pj],
                        scalar=idx_sb[:pj, j * K + k : j * K + k + 1],
                        in1=counts[:pj],
                        op0=mybir.AluOpType.is_equal,
                        op1=mybir.AluOpType.add,
                    )

            def emit_countsT(j, pj):
                counts = cntpool.tile([P, N], F32, tag="cnt")
                emit_hist(j, counts, pj)
                pst = ptr.tile([P, P], F32, tag="pst")
                nc.tensor.transpose(
                    pst[:, :pj], counts[:pj, :], ident_sb[:pj, :pj]
                )
                ct = ctpool.tile([P, P], BF16, tag="ct")
                nc.scalar.activation(
                    ct[:, :pj], pst[:, :pj], mybir.ActivationFunctionType.Copy
                )
                return ct

            # Partial block (64 rows, half-width DMAs) first so its stores
            # overlap the full-width stream instead of trailing it.
            order = [NBLK - 1] + list(range(NBLK - 1))
            cts = [None] * NBLK
            ncopy = 0
            for cc in range(CC):
                for j in order:
                    pj = min(P, ROWS - j * P)
                    if cc == 0:
                        cts[j] = emit_countsT(j, pj)
                    ct = cts[j]
                    ot = wpool.tile([P, CW], F32, tag="ot")
                    for h in range(2):
                        ps = pmm.tile([P, 2 * FD], F32, tag="ps")
                        for q in range(2):
                            nc.tensor.matmul(
                                ps[:pj, q * FD : (q + 1) * FD],
                                ct[:, :pj],
                                Wt[cc][:, h * 2 * FD + q * FD : h * 2 * FD + (q + 1) * FD],
                                start=True,
                                stop=True,
                            )
                        dst = ot[:pj, h * 2 * FD : (h + 1) * 2 * FD]
                        if ncopy % 2 == 0:
                            nc.scalar.activation(
                                dst, ps[:pj], mybir.ActivationFunctionType.Copy
                            )
                        else:
                            nc.vector.tensor_copy(out=dst, in_=ps[:pj])
                        ncopy += 1
                    nc.sync.dma_start(
                        out=out[j * P : j * P + pj, cc * CW : (cc + 1) * CW],
                        in_=ot[:pj],
                    )

    nc.finalize()
    return nc


def _get_nc():
    if "nc" not in _NC_CACHE:
        _NC_CACHE["nc"] = _build_nc()
    return _NC_CACHE["nc"]


def _prepare_in_maps(concepts, emb_table):
    concepts = np.asarray(concepts)
    emb = np.ascontiguousarray(np.asarray(emb_table, dtype=np.float32))

    # per-core index shards, padded to NBLK*P rows, laid out [P, NBLK*K]
    conc = concepts.reshape(NCORES, ROWS, K).astype(np.float32)
    idx_pad = np.full((NCORES, NBLK * P, K), float(N), dtype=np.float32)
    idx_pad[:, :ROWS] = conc
    # [core, NBLK, P, K] -> [core, P, NBLK*K]
    idx_dev = np.ascontiguousarray(
        idx_pad.reshape(NCORES, NBLK, P, K).transpose(0, 2, 1, 3).reshape(NCORES, P, NBLK * K)
    )

    iota = np.ascontiguousarray(
        np.broadcast_to(np.arange(N, dtype=np.float32), (P, N))
    )
    ident = np.ascontiguousarray(np.eye(P, dtype=np.float32))
    return [
        {"idx": idx_dev[i], "emb": emb, "iota": iota, "ident": ident}
        for i in range(NCORES)
    ]


def _run(concepts, emb_table, **spmd_kwargs):
    nc = _get_nc()
    in_maps = _prepare_in_maps(concepts, emb_table)
    res = run_bass_kernel_spmd(nc, in_maps, core_ids=list(range(NCORES)), **spmd_kwargs)
    out = np.concatenate(
        [res.results[i]["out"].reshape(B_LOC, S, N, D) for i in range(NCORES)],
        axis=0,
    )
    return out, res


def kernel(concepts, emb_table):
    out, _ = _run(concepts, emb_table)
    return out
